# Initial kernel scaffold
#
"""Trainium2 Bass kernel for nn_MixtureOfExperts_72035191488929.

Strategy (expert-parallel, top-2 routing, 8 NeuronCores):
  - Each core owns one expert's W1/W2 (bf16, pre-tiled on host).
  - Gating is computed in fp32, sharded over cores (512 tokens each), and the
    per-token routing info (top-2 indices + renormalized gates) is AllGathered.
  - Each core compacts the token ids routed to its expert via a matmul-based
    prefix sum + indirect-DMA scatter, gathers those rows of x (bf16) with an
    indirect DMA, runs the two GEMMs (relu MLP) on the PE array in bf16, and
    scales the result rows by the combine gate.
  - Each core returns its compacted, scaled expert outputs plus the token ids;
    the host unshards by adding each core's rows into the full [B, D] output
    (ids within one core are unique; each token appears on exactly 2 cores).

Self-contained: hardcodes all shapes from the problem spec.
"""

import numpy as np
import ml_dtypes

# ---------------------------------------------------------------------------
# Problem constants
# ---------------------------------------------------------------------------
B, D, H, E, K = 4096, 3072, 4096, 8, 2
N_CORES = 8
P = 128
SHARD = B // N_CORES          # 512 tokens gated per core
CAP = 1280                    # per-expert token capacity (mean 1024, sigma ~28)
NBLK = CAP // P               # 10 compact row blocks
CB = [(0, 512), (512, 512), (1024, 256)]   # token column blocks for the GEMMs
KD = D // P                   # 24 contraction chunks for GEMM1
KH = H // P                   # 32 contraction chunks for GEMM2
MH = H // P                   # 32 output chunks for GEMM1
MD = D // P                   # 24 output chunks for GEMM2
NTILES = B // P               # 32 token tiles for routing
DUMP = B                      # x row index used for padded slots (zero row)

# ---------------------------------------------------------------------------
# Workaround for this neuronxcc/walrus build: an instruction may carry at most
# one embedded semaphore wait ("Too many sync wait commands" otherwise).
# Tile's kernel-tail drain gets one wait per live DMA semaphore; split them
# across standalone SP nops.
# ---------------------------------------------------------------------------
_PATCHED = False


def _install_tile_patch():
    global _PATCHED
    if _PATCHED:
        return
    import bass_rust
    import concourse.tile as tile_mod
    from concourse.vector_clock import ScopedClock

    def _drain_and_barrier(self, tick_clock, wait_clock):
        nc = self.nc
        probe = nc.sync.nop(nofuse=True)
        wait_clock.add_sem_waits(
            probe.ins, ScopedClock({None: tick_clock.global_clock})
        )
        si = probe.ins.sync_info
        waits = list(si.on_wait) if si is not None and si.on_wait else []
        if len(waits) > 1:
            si.on_wait = [waits[0]]
            for w in waits[1:]:
                n2 = nc.sync.nop(nofuse=True)
                si2 = n2.ins.sync_info
                if si2 is None:
                    n2.ins.sync_info = bass_rust.SyncInfo(on_wait=[w], on_update=[])
                else:
                    si2.on_wait = [w]
        nc.sync.drain()
        nc.all_engine_barrier()
        assert self.sems is not None
        popped = nc._tile_sem_poison_stack.pop()
        assert popped is self._sem_poison
        nc.clear_and_free_semaphores(list(self.sems.allocated().values()))
        nc.all_engine_barrier()

    tile_mod.TileContext._drain_and_barrier = _drain_and_barrier
    _PATCHED = True


# ---------------------------------------------------------------------------
# Device program
# ---------------------------------------------------------------------------
_PROG = None


def build_program():
    """Build the SPMD Bass program (one program, 8 cores with different data)."""
    _install_tile_patch()
    import concourse.bass as bass
    import concourse.mybir as mybir
    import concourse.tile as tile
    from concourse.bass import IndirectOffsetOnAxis
    from concourse.masks import make_identity

    f32 = mybir.dt.float32
    bf16 = mybir.dt.bfloat16
    i32 = mybir.dt.int32
    u32 = mybir.dt.uint32
    AX = mybir.AxisListType
    OP = mybir.AluOpType
    AF = mybir.ActivationFunctionType

    nc = bass.Bass("TRN2", target_bir_lowering=False, debug=False,
                   num_devices=N_CORES)

    # Inputs (per core)
    xs_d = nc.dram_tensor("xs", [SHARD, D], f32, kind="ExternalInput")
    xb_d = nc.dram_tensor("xb", [B + 1, D], bf16, kind="ExternalInput")
    wg_d = nc.dram_tensor("wg", [D, E], f32, kind="ExternalInput")
    bg_d = nc.dram_tensor("bg", [1, E], f32, kind="ExternalInput")
    me_d = nc.dram_tensor("myexp", [1, 1], f32, kind="ExternalInput")
    w1_d = nc.dram_tensor("w1t", [MH, P, KD, P], bf16, kind="ExternalInput")
    w2_d = nc.dram_tensor("w2t", [MD, P, KH, P], bf16, kind="ExternalInput")
    b1_d = nc.dram_tensor("b1r", [P, MH], f32, kind="ExternalInput")
    b2_d = nc.dram_tensor("b2r", [P, MD], f32, kind="ExternalInput")
    su128_d = nc.dram_tensor("su128", [P, P], f32, kind="ExternalInput")
    su32_d = nc.dram_tensor("su32", [32, 32], f32, kind="ExternalInput")

    # Outputs (per core)
    probs_d = nc.dram_tensor("probs", [SHARD, E], f32, kind="ExternalOutput")
    topk_d = nc.dram_tensor("topk", [SHARD, K], i32, kind="ExternalOutput")
    yids_d = nc.dram_tensor("yids", [CAP, 1], i32, kind="ExternalOutput")
    ycomp_d = nc.dram_tensor("ycomp", [CAP, D], bf16, kind="ExternalOutput")

    with tile.TileContext(nc) as tc:
        with (
            tc.tile_pool(name="sb", bufs=1) as sb,
            tc.tile_pool(name="ps", bufs=1, space="PSUM") as ps,
            tc.tile_pool(name="dr", bufs=1, space="DRAM") as dr,
        ):
            # ---------------- constants ----------------
            id128f = sb.tile([P, P], f32, tag="c_idf")
            make_identity(nc, id128f[:])
            id128b = sb.tile([P, P], bf16, tag="c_idb")
            nc.vector.tensor_copy(out=id128b[:], in_=id128f[:])
            su128_sb = sb.tile([P, P], f32, tag="c_su")
            nc.sync.dma_start(out=su128_sb[:], in_=su128_d[:])
            su32_sb = sb.tile([32, 32], f32, tag="c_su32")
            nc.sync.dma_start(out=su32_sb[:], in_=su32_d[:])
            wg_sb = sb.tile([P, KD, E], f32, tag="c_wg")
            nc.sync.dma_start(
                out=wg_sb[:], in_=wg_d[:].rearrange("(k d) e -> d k e", d=P)
            )
            bg_sb = sb.tile([1, E], f32, tag="c_bg")
            nc.sync.dma_start(out=bg_sb[:], in_=bg_d[:])
            me_sb = sb.tile([1, 1], f32, tag="c_me")
            nc.sync.dma_start(out=me_sb[:], in_=me_d[:])
            b1_sb = sb.tile([P, MH], f32, tag="c_b1")
            nc.sync.dma_start(out=b1_sb[:], in_=b1_d[:])
            b2_sb = sb.tile([P, MD], f32, tag="c_b2")
            nc.sync.dma_start(out=b2_sb[:], in_=b2_d[:])
            ones1_128 = sb.tile([1, P], f32, tag="c_o1")
            nc.vector.memset(ones1_128[:], 1.0)
            ones128_1 = sb.tile([P, 1], f32, tag="c_o2")
            nc.vector.memset(ones128_1[:], 1.0)
            one1 = sb.tile([1, 1], f32, tag="c_o3")
            nc.vector.memset(one1[:], 1.0)

            # broadcast bg and my-expert id across partitions via K=1 matmul
            bgbc_ps = ps.tile([P, E], f32, tag="mm", bufs=6)
            nc.tensor.matmul(out=bgbc_ps[:], lhsT=ones1_128[:], rhs=bg_sb[:],
                             skip_group_check=True)
            bgbc = sb.tile([P, E], f32, tag="c_bgbc")
            nc.vector.tensor_copy(out=bgbc[:], in_=bgbc_ps[:])
            mebc_ps = ps.tile([P, 1], f32, tag="mm", bufs=6)
            nc.tensor.matmul(out=mebc_ps[:], lhsT=ones1_128[:], rhs=me_sb[:],
                             skip_group_check=True)
            mebc = sb.tile([P, 1], f32, tag="c_mebc")
            nc.vector.tensor_copy(out=mebc[:], in_=mebc_ps[:])

            # DRAM scratch
            ccin = dr.tile([SHARD, 4], f32, tag="ccin")
            ccout = dr.tile([B, 4], f32, tag="ccout", addr_space="Shared")
            pk = dr.tile([11 * P, 2], f32, tag="pk")

            # ---------------- phase A: gating on this core's shard ----------
            for t in range(SHARD // P):
                lg_ps = ps.tile([P, E], f32, tag="mm", bufs=6)
                for half in range(2):
                    xs_h = sb.tile([P, D // 2], f32, tag="xbuf", bufs=2)
                    nc.sync.dma_start(
                        out=xs_h[:],
                        in_=xs_d[t * P:(t + 1) * P,
                                 half * (D // 2):(half + 1) * (D // 2)],
                    )
                    for kk in range(KD // 2):
                        k = half * (KD // 2) + kk
                        tp_ps = ps.tile([P, P], f32, tag="tp", bufs=2)
                        nc.tensor.transpose(
                            out=tp_ps[:], in_=xs_h[:, kk * P:(kk + 1) * P],
                            identity=id128f[:],
                        )
                        xsT = sb.tile([P, P], f32, tag="xsT", bufs=3)
                        nc.vector.tensor_copy(out=xsT[:], in_=tp_ps[:])
                        nc.tensor.matmul(
                            out=lg_ps[:], lhsT=xsT[:], rhs=wg_sb[:, k, :],
                            start=(k == 0), stop=(k == KD - 1),
                            skip_group_check=True,
                        )
                logits = sb.tile([P, E], f32, tag="lgt", bufs=2)
                nc.vector.tensor_add(out=logits[:], in0=lg_ps[:], in1=bgbc[:])

                # full softmax (gate_probs output)
                umax = sb.tile([P, 8], f32, tag="umax", bufs=2)
                nc.vector.max(out=umax[:], in_=logits[:])
                uidx = sb.tile([P, 8], u32, tag="uidx", bufs=2)
                nc.vector.max_index(out=uidx[:], in_max=umax[:], in_values=logits[:])
                shf = sb.tile([P, E], f32, tag="shf", bufs=2)
                nc.vector.tensor_scalar(
                    out=shf[:], in0=logits[:], scalar1=umax[:, 0:1], scalar2=None,
                    op0=OP.subtract,
                )
                expd = sb.tile([P, E], f32, tag="expd", bufs=2)
                nc.scalar.activation(out=expd[:], in_=shf[:], func=AF.Exp)
                ssum = sb.tile([P, 1], f32, tag="ssum", bufs=2)
                nc.vector.tensor_reduce(out=ssum[:], in_=expd[:], axis=AX.X,
                                        op=OP.add)
                rinv = sb.tile([P, 1], f32, tag="rinv", bufs=2)
                nc.vector.reciprocal(out=rinv[:], in_=ssum[:])
                prb = sb.tile([P, E], f32, tag="prb", bufs=2)
                nc.vector.tensor_scalar(
                    out=prb[:], in0=expd[:], scalar1=rinv[:, 0:1], scalar2=None,
                    op0=OP.mult,
                )
                nc.sync.dma_start(out=probs_d[t * P:(t + 1) * P, :], in_=prb[:])

                # top-2 indices output
                tki = sb.tile([P, K], i32, tag="tki", bufs=2)
                nc.vector.tensor_copy(out=tki[:], in_=uidx[:, 0:K])
                nc.sync.dma_start(out=topk_d[t * P:(t + 1) * P, :], in_=tki[:])

                # top-2 renormalized gates: g1 = 1/(1+e), g2 = e/(1+e),
                # e = exp(l2 - l1)
                dlt = sb.tile([P, 1], f32, tag="dlt", bufs=2)
                nc.vector.tensor_sub(out=dlt[:], in0=umax[:, 1:2], in1=umax[:, 0:1])
                ex2 = sb.tile([P, 1], f32, tag="ex2", bufs=2)
                nc.scalar.activation(out=ex2[:], in_=dlt[:], func=AF.Exp)
                s2 = sb.tile([P, 1], f32, tag="s2", bufs=2)
                nc.vector.tensor_scalar(out=s2[:], in0=ex2[:], scalar1=1.0,
                                        scalar2=None, op0=OP.add)
                g1 = sb.tile([P, 1], f32, tag="g1", bufs=2)
                nc.vector.reciprocal(out=g1[:], in_=s2[:])
                g2 = sb.tile([P, 1], f32, tag="g2", bufs=2)
                nc.vector.tensor_mul(out=g2[:], in0=ex2[:], in1=g1[:])

                # pack (i1, i2, g1, g2) and stage for the AllGather
                rtp = sb.tile([P, 4], f32, tag="rtp", bufs=2)
                nc.vector.tensor_copy(out=rtp[:, 0:1], in_=uidx[:, 0:1])
                nc.vector.tensor_copy(out=rtp[:, 1:2], in_=uidx[:, 1:2])
                nc.vector.tensor_copy(out=rtp[:, 2:3], in_=g1[:])
                nc.vector.tensor_copy(out=rtp[:, 3:4], in_=g2[:])
                nc.sync.dma_start(out=ccin[t * P:(t + 1) * P, :], in_=rtp[:])

            # ---------------- phase B: allgather routing info ---------------
            nc.gpsimd.collective_compute(
                "AllGather", OP.bypass,
                replica_groups=[list(range(N_CORES))],
                ins=[ccin[:]], outs=[ccout[:]],
            )
            rt = sb.tile([P, NTILES, 4], f32, tag="rt")
            nc.sync.dma_start(
                out=rt[:], in_=ccout[:].rearrange("(i p) c -> p i c", p=P)
            )

            # ---------------- phase C: routing masks + prefix positions -----
            m1 = sb.tile([P, NTILES], f32, tag="m1")
            nc.vector.tensor_scalar(out=m1[:], in0=rt[:, :, 0], scalar1=mebc[:, 0:1],
                                    scalar2=None, op0=OP.is_equal)
            m2 = sb.tile([P, NTILES], f32, tag="m2")
            nc.vector.tensor_scalar(out=m2[:], in0=rt[:, :, 1], scalar1=mebc[:, 0:1],
                                    scalar2=None, op0=OP.is_equal)
            msk = sb.tile([P, NTILES], f32, tag="msk")
            nc.vector.tensor_add(out=msk[:], in0=m1[:], in1=m2[:])

            iw = sb.tile([P, NTILES, 2], f32, tag="iw")
            nc.gpsimd.iota(out=iw[:, :, 0], pattern=[[P, NTILES]], base=0,
                           channel_multiplier=1,
                           allow_small_or_imprecise_dtypes=True)
            wa = sb.tile([P, NTILES], f32, tag="wa")
            nc.vector.tensor_mul(out=wa[:], in0=m1[:], in1=rt[:, :, 2])
            wb = sb.tile([P, NTILES], f32, tag="wb")
            nc.vector.tensor_mul(out=wb[:], in0=m2[:], in1=rt[:, :, 3])
            nc.vector.tensor_add(out=iw[:, :, 1], in0=wa[:], in1=wb[:])

            # within-column exclusive prefix (over partitions)
            pp_ps = ps.tile([P, NTILES], f32, tag="mm", bufs=6)
            nc.tensor.matmul(out=pp_ps[:], lhsT=su128_sb[:], rhs=msk[:],
                             skip_group_check=True)
            # column sums -> [1, 32]
            cs_ps = ps.tile([1, NTILES], f32, tag="mm", bufs=6)
            nc.tensor.matmul(out=cs_ps[:], lhsT=ones128_1[:], rhs=msk[:],
                             skip_group_check=True)
            cs_sb = sb.tile([1, NTILES], f32, tag="cs")
            nc.vector.tensor_copy(out=cs_sb[:], in_=cs_ps[:])
            # transpose to [32, 1]
            csT_ps = ps.tile([NTILES, 1], f32, tag="mm", bufs=6)
            nc.tensor.matmul(out=csT_ps[:], lhsT=cs_sb[:], rhs=one1[:],
                             skip_group_check=True)
            csT_sb = sb.tile([NTILES, 1], f32, tag="csT")
            nc.vector.tensor_copy(out=csT_sb[:], in_=csT_ps[:])
            # exclusive prefix over the 32 column sums
            cp_ps = ps.tile([NTILES, 1], f32, tag="mm", bufs=6)
            nc.tensor.matmul(out=cp_ps[:], lhsT=su32_sb[:], rhs=csT_sb[:],
                             skip_group_check=True)
            cp_sb = sb.tile([NTILES, 1], f32, tag="cp")
            nc.vector.tensor_copy(out=cp_sb[:], in_=cp_ps[:])
            # transpose back to a row [1, 32]
            cpr_ps = ps.tile([1, NTILES], f32, tag="mm", bufs=6)
            nc.tensor.transpose(out=cpr_ps[:], in_=cp_sb[:],
                                identity=id128f[:NTILES, :NTILES])
            cpr_sb = sb.tile([1, NTILES], f32, tag="cpr")
            nc.vector.tensor_copy(out=cpr_sb[:], in_=cpr_ps[:])
            # broadcast over partitions
            cpb_ps = ps.tile([P, NTILES], f32, tag="mm", bufs=6)
            nc.tensor.matmul(out=cpb_ps[:], lhsT=ones1_128[:], rhs=cpr_sb[:],
                             skip_group_check=True)
            cpb_sb = sb.tile([P, NTILES], f32, tag="cpb")
            nc.vector.tensor_copy(out=cpb_sb[:], in_=cpb_ps[:])

            pos = sb.tile([P, NTILES], f32, tag="pos")
            nc.vector.tensor_add(out=pos[:], in0=pp_ps[:], in1=cpb_sb[:])
            # non-selected tokens -> slot CAP (dump row); clamp overflow to CAP
            pos1 = sb.tile([P, NTILES], f32, tag="pos1")
            nc.vector.tensor_scalar(out=pos1[:], in0=pos[:], scalar1=float(CAP),
                                    scalar2=None, op0=OP.subtract)
            pos2 = sb.tile([P, NTILES], f32, tag="pos2")
            nc.vector.tensor_mul(out=pos2[:], in0=pos1[:], in1=msk[:])
            pos3 = sb.tile([P, NTILES], f32, tag="pos3")
            nc.vector.tensor_scalar(out=pos3[:], in0=pos2[:], scalar1=float(CAP),
                                    scalar2=float(CAP), op0=OP.add, op1=OP.min)
            posi = sb.tile([P, NTILES], i32, tag="posi")
            nc.vector.tensor_copy(out=posi[:], in_=pos3[:])

            # ---------------- phase D: compact (id, gate) via scatter -------
            pf = sb.tile([P, 2], f32, tag="pf")
            nc.vector.memset(pf[:, 0:1], float(DUMP))
            nc.vector.memset(pf[:, 1:2], 0.0)
            for j in range(11):
                nc.sync.dma_start(out=pk[j * P:(j + 1) * P, :], in_=pf[:])
            for i in range(NTILES):
                nc.gpsimd.indirect_dma_start(
                    out=pk[:],
                    out_offset=IndirectOffsetOnAxis(ap=posi[:, i:i + 1], axis=0),
                    in_=iw[:, i, :],
                    in_offset=None,
                )
            pkt = sb.tile([P, NBLK, 2], f32, tag="pkt")
            nc.sync.dma_start(
                out=pkt[:], in_=pk[0:CAP, :].rearrange("(b p) c -> p b c", p=P)
            )
            idsi = sb.tile([P, NBLK], i32, tag="idsi")
            nc.vector.tensor_copy(out=idsi[:], in_=pkt[:, :, 0])
            nc.sync.dma_start(
                out=yids_d[:].rearrange("(b p) c -> p b c", p=P),
                in_=idsi[:, :, None],
            )

            # ---------------- phase F: gather x rows + transpose ------------
            xeT = sb.tile([P, KD, CAP], bf16, tag="xeT")
            for b in range(NBLK):
                xg = sb.tile([P, D], bf16, tag="xbuf", bufs=2)
                nc.gpsimd.indirect_dma_start(
                    out=xg[:],
                    out_offset=None,
                    in_=xb_d[:],
                    in_offset=IndirectOffsetOnAxis(ap=idsi[:, b:b + 1], axis=0),
                )
                for k in range(KD):
                    tpb = ps.tile([P, P], bf16, tag="tp", bufs=2)
                    nc.tensor.transpose(
                        out=tpb[:], in_=xg[:, k * P:(k + 1) * P],
                        identity=id128b[:],
                    )
                    nc.scalar.copy(out=xeT[:, k, b * P:(b + 1) * P], in_=tpb[:])

            # ---------------- phase G: GEMM1 (h = relu(x W1 + b1)) ----------
            hT = sb.tile([P, KH, CAP], bf16, tag="hT")
            for m in range(MH):
                w1s = sb.tile([P, KD, P], bf16, tag="wstrip", bufs=2)
                nc.sync.dma_start(out=w1s[:], in_=w1_d[m])
                h_ps = [
                    ps.tile([P, W], f32, tag="mm", bufs=6, name=f"hps{m}_{cb}")
                    for cb, (c0, W) in enumerate(CB)
                ]
                for k in range(KD):
                    for cb, (c0, W) in enumerate(CB):
                        nc.tensor.matmul(
                            out=h_ps[cb][:], lhsT=w1s[:, k, :],
                            rhs=xeT[:, k, c0:c0 + W],
                            start=(k == 0), stop=(k == KD - 1),
                            skip_group_check=True,
                        )
                for cb, (c0, W) in enumerate(CB):
                    nc.scalar.activation(
                        out=hT[:, m, c0:c0 + W], in_=h_ps[cb][:], func=AF.Relu,
                        bias=b1_sb[:, m:m + 1], scale=1.0,
                    )

            # ---------------- phase H/I: GEMM2 + gate-scale + emit ----------
            for cb, (c0, W) in enumerate(CB):
                nt4 = W // P
                yrow = [
                    sb.tile([P, D], bf16, tag="yrow", bufs=4, name=f"yr{cb}_{t4}")
                    for t4 in range(nt4)
                ]
                for mo in range(MD):
                    w2s = sb.tile([P, KH, P], bf16, tag="wstrip", bufs=2)
                    nc.sync.dma_start(out=w2s[:], in_=w2_d[mo])
                    y_ps = ps.tile([P, W], f32, tag="mm", bufs=6)
                    for k in range(KH):
                        nc.tensor.matmul(
                            out=y_ps[:], lhsT=w2s[:, k, :],
                            rhs=hT[:, k, c0:c0 + W],
                            start=(k == 0), stop=(k == KH - 1),
                            skip_group_check=True,
                        )
                    yT = sb.tile([P, W], f32, tag="yt", bufs=2)
                    nc.scalar.activation(
                        out=yT[:], in_=y_ps[:], func=AF.Identity,
                        bias=b2_sb[:, mo:mo + 1], scale=1.0,
                    )
                    for t4 in range(nt4):
                        bb = cb * 4 + t4
                        tpy = ps.tile([P, P], f32, tag="tp", bufs=2)
                        nc.tensor.transpose(
                            out=tpy[:], in_=yT[:, t4 * P:(t4 + 1) * P],
                            identity=id128f[:],
                        )
                        nc.vector.tensor_scalar(
                            out=yrow[t4][:, mo * P:(mo + 1) * P], in0=tpy[:],
                            scalar1=pkt[:, bb, 1:2], scalar2=None, op0=OP.mult,
                        )
                for t4 in range(nt4):
                    bb = cb * 4 + t4
                    nc.sync.dma_start(
                        out=ycomp_d[bb * P:(bb + 1) * P, :], in_=yrow[t4][:]
                    )

    return nc


def _get_prog():
    global _PROG
    if _PROG is None:
        _PROG = build_program()
    return _PROG


# ---------------------------------------------------------------------------
# Host-side sharding / unsharding
# ---------------------------------------------------------------------------
def make_in_maps(x, Wg, bg, W1, b1, W2, b2):
    bf16 = ml_dtypes.bfloat16
    x = np.ascontiguousarray(np.asarray(x, dtype=np.float32))
    Wg = np.ascontiguousarray(np.asarray(Wg, dtype=np.float32))
    bg = np.asarray(bg, dtype=np.float32).reshape(1, E)
    W1 = np.asarray(W1, dtype=np.float32)
    b1 = np.asarray(b1, dtype=np.float32)
    W2 = np.asarray(W2, dtype=np.float32)
    b2 = np.asarray(b2, dtype=np.float32)

    xb = np.concatenate([x, np.zeros((1, D), np.float32)], axis=0).astype(bf16)
    su128 = (np.arange(P)[:, None] < np.arange(P)[None, :]).astype(np.float32)
    su32 = (np.arange(32)[:, None] < np.arange(32)[None, :]).astype(np.float32)

    in_maps = []
    for e in range(N_CORES):
        w1e = W1[e].reshape(KD, P, MH, P).transpose(2, 1, 0, 3)  # [MH,P(d),KD,P(h)]
        w2e = W2[e].reshape(KH, P, MD, P).transpose(2, 1, 0, 3)  # [MD,P(h),KH,P(d)]
        in_maps.append({
            "xs": np.ascontiguousarray(x[e * SHARD:(e + 1) * SHARD]),
            "xb": xb,
            "wg": Wg,
            "bg": bg,
            "myexp": np.array([[float(e)]], np.float32),
            "w1t": np.ascontiguousarray(w1e).astype(bf16),
            "w2t": np.ascontiguousarray(w2e).astype(bf16),
            "b1r": np.ascontiguousarray(b1[e].reshape(MH, P).T),
            "b2r": np.ascontiguousarray(b2[e].reshape(MD, P).T),
            "su128": su128,
            "su32": su32,
        })
    return in_maps


def assemble_outputs(results):
    probs = np.concatenate([np.asarray(r["probs"]) for r in results], axis=0)
    topk = np.concatenate([np.asarray(r["topk"]) for r in results], axis=0)
    out = np.zeros((B + 1, D), np.float32)
    for r in results:
        ids = np.asarray(r["yids"]).reshape(-1)
        out[ids] += np.asarray(r["ycomp"]).astype(np.float32)
    return out[:B], probs, topk.astype(np.int32)


def kernel(x, Wg, bg, W1, b1, W2, b2):
    from concourse.bass_utils import run_bass_kernel_spmd

    nc = _get_prog()
    in_maps = make_in_maps(x, Wg, bg, W1, b1, W2, b2)
    res = run_bass_kernel_spmd(nc, in_maps, core_ids=list(range(N_CORES)))
    return assemble_outputs(res.results)


# revision 10
# speedup vs baseline: 42.9415x; 42.9415x over previous
"""Trainium2 Bass kernel for nn_MixtureOfExperts_72035191488929.

Strategy (expert-parallel, top-2 routing, 8 NeuronCores):
  - Each core owns one expert's W1/W2 (bf16, pre-tiled on host).
  - Gating is computed in fp32, sharded over cores (512 tokens each), and the
    per-token routing info (top-2 indices + renormalized gates) is AllGathered.
  - Each core compacts the token ids routed to its expert via a matmul-based
    prefix sum + indirect-DMA scatter, gathers those rows of x (bf16) with an
    indirect DMA, runs the two GEMMs (relu MLP) on the PE array in bf16, and
    scales the result rows by the combine gate.
  - Each core returns its compacted, scaled expert outputs plus the token ids;
    the host unshards by adding each core's rows into the full [B, D] output
    (ids within one core are unique; each token appears on exactly 2 cores).

Self-contained: hardcodes all shapes from the problem spec.
"""

import numpy as np
import ml_dtypes

# ---------------------------------------------------------------------------
# Problem constants
# ---------------------------------------------------------------------------
B, D, H, E, K = 4096, 3072, 4096, 8, 2
N_CORES = 8
P = 128
SHARD = B // N_CORES          # 512 tokens gated per core
CAP = 1280                    # per-expert token capacity (mean 1024, sigma ~28)
NBLK = CAP // P               # 10 compact row blocks
CB = [(0, 512), (512, 512), (1024, 256)]   # token column blocks for the GEMMs
KD = D // P                   # 24 contraction chunks for GEMM1
KH = H // P                   # 32 contraction chunks for GEMM2
MH = H // P                   # 32 output chunks for GEMM1
MD = D // P                   # 24 output chunks for GEMM2
NTILES = B // P               # 32 token tiles for routing
DUMP = B                      # x row index used for padded slots (zero row)

# ---------------------------------------------------------------------------
# Workaround for this neuronxcc/walrus build: an instruction may carry at most
# one embedded semaphore wait ("Too many sync wait commands" otherwise).
# Tile's kernel-tail drain gets one wait per live DMA semaphore; split them
# across standalone SP nops.
# ---------------------------------------------------------------------------
_PATCHED = False


def _install_tile_patch():
    global _PATCHED
    if _PATCHED:
        return
    import bass_rust
    import concourse.mybir as mybir
    import concourse.tile as tile_mod
    from concourse.vector_clock import ScopedClock

    _orig_add = tile_mod.TileContext._add_instruction

    def _add_instruction(self, inst):
        si = inst.sync_info
        if si is not None and si.on_wait and len(si.on_wait) > 1:
            waits = list(si.on_wait)
            si.on_wait = [waits[-1]]
            for j, w in enumerate(waits[:-1]):
                nop = mybir.InstNoOp(name=f"{inst.name}-sw{j}", ins=[], outs=[])
                nop.engine = inst.engine
                nop.sync_info = bass_rust.SyncInfo(on_wait=[w], on_update=[])
                _orig_add(self, nop)
        _orig_add(self, inst)

    def _drain_and_barrier(self, tick_clock, wait_clock):
        nc = self.nc
        probe = nc.sync.nop(nofuse=True)
        wait_clock.add_sem_waits(
            probe.ins, ScopedClock({None: tick_clock.global_clock})
        )
        si = probe.ins.sync_info
        waits = list(si.on_wait) if si is not None and si.on_wait else []
        if len(waits) > 1:
            si.on_wait = [waits[0]]
            for w in waits[1:]:
                n2 = nc.sync.nop(nofuse=True)
                si2 = n2.ins.sync_info
                if si2 is None:
                    n2.ins.sync_info = bass_rust.SyncInfo(on_wait=[w], on_update=[])
                else:
                    si2.on_wait = [w]
        nc.sync.drain()
        nc.all_engine_barrier()
        assert self.sems is not None
        popped = nc._tile_sem_poison_stack.pop()
        assert popped is self._sem_poison
        nc.clear_and_free_semaphores(list(self.sems.allocated().values()))
        nc.all_engine_barrier()

    tile_mod.TileContext._add_instruction = _add_instruction
    tile_mod.TileContext._drain_and_barrier = _drain_and_barrier
    _PATCHED = True


# ---------------------------------------------------------------------------
# Device program
# ---------------------------------------------------------------------------
_PROG = None


def build_program():
    """Build the SPMD Bass program (one program, 8 cores with different data)."""
    import os
    skip = set(os.environ.get("MOE_SKIP", "").split(","))
    _install_tile_patch()
    import concourse.bass as bass
    import concourse.mybir as mybir
    import concourse.tile as tile
    from concourse.bass import IndirectOffsetOnAxis
    from concourse.masks import make_identity

    f32 = mybir.dt.float32
    bf16 = mybir.dt.bfloat16
    i32 = mybir.dt.int32
    u32 = mybir.dt.uint32
    AX = mybir.AxisListType
    OP = mybir.AluOpType
    AF = mybir.ActivationFunctionType

    nc = bass.Bass("TRN2", target_bir_lowering=False, debug=False,
                   num_devices=N_CORES)

    # Inputs (per core)
    xs_d = nc.dram_tensor("xs", [SHARD, D], f32, kind="ExternalInput")
    xb_d = nc.dram_tensor("xb", [B + 1, D], bf16, kind="ExternalInput")
    wg_d = nc.dram_tensor("wg", [D, E], f32, kind="ExternalInput")
    bg_d = nc.dram_tensor("bg", [1, E], f32, kind="ExternalInput")
    me_d = nc.dram_tensor("myexp", [1, 1], f32, kind="ExternalInput")
    w1_d = nc.dram_tensor("w1t", [MH, P, KD, P], bf16, kind="ExternalInput")
    w2_d = nc.dram_tensor("w2t", [MD, P, KH, P], bf16, kind="ExternalInput")
    b1_d = nc.dram_tensor("b1r", [P, MH], f32, kind="ExternalInput")
    b2_d = nc.dram_tensor("b2r", [P, MD], f32, kind="ExternalInput")
    su128_d = nc.dram_tensor("su128", [P, P], f32, kind="ExternalInput")
    su32_d = nc.dram_tensor("su32", [32, 32], f32, kind="ExternalInput")

    # Outputs (per core)
    probs_d = nc.dram_tensor("probs", [SHARD, E], f32, kind="ExternalOutput")
    topk_d = nc.dram_tensor("topk", [SHARD, K], i32, kind="ExternalOutput")
    yids_d = nc.dram_tensor("yids", [CAP, 1], i32, kind="ExternalOutput")
    ycomp_d = nc.dram_tensor("ycomp", [CAP, D], f32, kind="ExternalOutput")

    with tile.TileContext(nc) as tc:
        with (
            tc.tile_pool(name="sb", bufs=1) as sb,
            tc.tile_pool(name="ps", bufs=1, space="PSUM") as ps,
            tc.tile_pool(name="dr", bufs=1, space="DRAM") as dr,
        ):
            # ---------------- constants ----------------
            id128f = sb.tile([P, P], f32, tag="c_idf")
            make_identity(nc, id128f[:])
            id128b = sb.tile([P, P], bf16, tag="c_idb")
            nc.vector.tensor_copy(out=id128b[:], in_=id128f[:])
            su128_sb = sb.tile([P, P], f32, tag="c_su")
            nc.sync.dma_start(out=su128_sb[:], in_=su128_d[:])
            su32_sb = sb.tile([32, 32], f32, tag="c_su32")
            nc.sync.dma_start(out=su32_sb[:], in_=su32_d[:])
            wg_sb = sb.tile([P, KD, E], f32, tag="c_wg")
            nc.sync.dma_start(
                out=wg_sb[:], in_=wg_d[:].rearrange("(k d) e -> d k e", d=P)
            )
            bg_sb = sb.tile([1, E], f32, tag="c_bg")
            nc.sync.dma_start(out=bg_sb[:], in_=bg_d[:])
            me_sb = sb.tile([1, 1], f32, tag="c_me")
            nc.sync.dma_start(out=me_sb[:], in_=me_d[:])
            b1_sb = sb.tile([P, MH], f32, tag="c_b1")
            nc.sync.dma_start(out=b1_sb[:], in_=b1_d[:])
            b2_sb = sb.tile([P, MD], f32, tag="c_b2")
            nc.sync.dma_start(out=b2_sb[:], in_=b2_d[:])
            ones1_128 = sb.tile([1, P], f32, tag="c_o1")
            nc.vector.memset(ones1_128[:], 1.0)
            ones128_1 = sb.tile([P, 1], f32, tag="c_o2")
            nc.vector.memset(ones128_1[:], 1.0)
            one1 = sb.tile([1, 1], f32, tag="c_o3")
            nc.vector.memset(one1[:], 1.0)

            # broadcast bg and my-expert id across partitions via K=1 matmul
            bgbc_ps = ps.tile([P, E], f32, tag="mm", bufs=6)
            nc.tensor.matmul(out=bgbc_ps[:], lhsT=ones1_128[:], rhs=bg_sb[:],
                             skip_group_check=True)
            bgbc = sb.tile([P, E], f32, tag="c_bgbc")
            nc.vector.tensor_copy(out=bgbc[:], in_=bgbc_ps[:])
            mebc_ps = ps.tile([P, 1], f32, tag="mm", bufs=6)
            nc.tensor.matmul(out=mebc_ps[:], lhsT=ones1_128[:], rhs=me_sb[:],
                             skip_group_check=True)
            mebc = sb.tile([P, 1], f32, tag="c_mebc")
            nc.vector.tensor_copy(out=mebc[:], in_=mebc_ps[:])

            # DRAM scratch
            ccin = dr.tile([SHARD, 4], f32, tag="ccin")
            ccout = dr.tile([B, 4], f32, tag="ccout", addr_space="Shared")
            pk = dr.tile([11 * P, 2], f32, tag="pk")

            # ---------------- phase A: gating on this core's shard ----------
            for t in range(SHARD // P):
                lg_ps = ps.tile([P, E], f32, tag="mm", bufs=6)
                for half in range(2):
                    xs_h = sb.tile([P, D // 2], f32, tag="xbuf", bufs=2)
                    nc.sync.dma_start(
                        out=xs_h[:],
                        in_=xs_d[t * P:(t + 1) * P,
                                 half * (D // 2):(half + 1) * (D // 2)],
                    )
                    for kk in range(KD // 2):
                        k = half * (KD // 2) + kk
                        tp_ps = ps.tile([P, P], f32, tag="tp", bufs=2)
                        nc.tensor.transpose(
                            out=tp_ps[:], in_=xs_h[:, kk * P:(kk + 1) * P],
                            identity=id128f[:],
                        )
                        xsT = sb.tile([P, P], f32, tag="xsT", bufs=3)
                        nc.vector.tensor_copy(out=xsT[:], in_=tp_ps[:])
                        nc.tensor.matmul(
                            out=lg_ps[:], lhsT=xsT[:], rhs=wg_sb[:, k, :],
                            start=(k == 0), stop=(k == KD - 1),
                            skip_group_check=True,
                        )
                logits = sb.tile([P, E], f32, tag="lgt", bufs=2)
                nc.vector.tensor_add(out=logits[:], in0=lg_ps[:], in1=bgbc[:])

                # full softmax (gate_probs output)
                umax = sb.tile([P, 8], f32, tag="umax", bufs=2)
                nc.vector.max(out=umax[:], in_=logits[:])
                uidx = sb.tile([P, 8], u32, tag="uidx", bufs=2)
                nc.vector.max_index(out=uidx[:], in_max=umax[:], in_values=logits[:])
                shf = sb.tile([P, E], f32, tag="shf", bufs=2)
                nc.vector.tensor_scalar(
                    out=shf[:], in0=logits[:], scalar1=umax[:, 0:1], scalar2=None,
                    op0=OP.subtract,
                )
                expd = sb.tile([P, E], f32, tag="expd", bufs=2)
                nc.scalar.activation(out=expd[:], in_=shf[:], func=AF.Exp)
                ssum = sb.tile([P, 1], f32, tag="ssum", bufs=2)
                nc.vector.tensor_reduce(out=ssum[:], in_=expd[:], axis=AX.X,
                                        op=OP.add)
                rinv = sb.tile([P, 1], f32, tag="rinv", bufs=2)
                nc.vector.reciprocal(out=rinv[:], in_=ssum[:])
                prb = sb.tile([P, E], f32, tag="prb", bufs=2)
                nc.vector.tensor_scalar(
                    out=prb[:], in0=expd[:], scalar1=rinv[:, 0:1], scalar2=None,
                    op0=OP.mult,
                )
                nc.sync.dma_start(out=probs_d[t * P:(t + 1) * P, :], in_=prb[:])

                # top-2 indices output
                tki = sb.tile([P, K], i32, tag="tki", bufs=2)
                nc.vector.tensor_copy(out=tki[:], in_=uidx[:, 0:K])
                nc.sync.dma_start(out=topk_d[t * P:(t + 1) * P, :], in_=tki[:])

                # top-2 renormalized gates: g1 = 1/(1+e), g2 = e/(1+e),
                # e = exp(l2 - l1)
                dlt = sb.tile([P, 1], f32, tag="dlt", bufs=2)
                nc.vector.tensor_sub(out=dlt[:], in0=umax[:, 1:2], in1=umax[:, 0:1])
                ex2 = sb.tile([P, 1], f32, tag="ex2", bufs=2)
                nc.scalar.activation(out=ex2[:], in_=dlt[:], func=AF.Exp)
                s2 = sb.tile([P, 1], f32, tag="s2", bufs=2)
                nc.vector.tensor_scalar(out=s2[:], in0=ex2[:], scalar1=1.0,
                                        scalar2=None, op0=OP.add)
                g1 = sb.tile([P, 1], f32, tag="g1", bufs=2)
                nc.vector.reciprocal(out=g1[:], in_=s2[:])
                g2 = sb.tile([P, 1], f32, tag="g2", bufs=2)
                nc.vector.tensor_mul(out=g2[:], in0=ex2[:], in1=g1[:])

                # pack (i1, i2, g1, g2) and stage for the AllGather
                rtp = sb.tile([P, 4], f32, tag="rtp", bufs=2)
                nc.vector.tensor_copy(out=rtp[:, 0:1], in_=uidx[:, 0:1])
                nc.vector.tensor_copy(out=rtp[:, 1:2], in_=uidx[:, 1:2])
                nc.vector.tensor_copy(out=rtp[:, 2:3], in_=g1[:])
                nc.vector.tensor_copy(out=rtp[:, 3:4], in_=g2[:])
                nc.sync.dma_start(out=ccin[t * P:(t + 1) * P, :], in_=rtp[:])

            # ---------------- phase B: allgather routing info ---------------
            rt = sb.tile([P, NTILES, 4], f32, tag="rt")
            if "ag" in skip:
                nc.sync.dma_start(
                    out=rt[:, :4, :],
                    in_=ccin[:].rearrange("(i p) c -> p i c", p=P),
                )
                nc.vector.memset(rt[:, 4:, :], 0.0)
            else:
                nc.gpsimd.collective_compute(
                    "AllGather", OP.bypass,
                    replica_groups=[list(range(N_CORES))],
                    ins=[ccin[:]], outs=[ccout[:]],
                )
                nc.sync.dma_start(
                    out=rt[:], in_=ccout[:].rearrange("(i p) c -> p i c", p=P)
                )

            # ---------------- phase C: routing masks + prefix positions -----
            m1 = sb.tile([P, NTILES], f32, tag="m1")
            nc.vector.tensor_scalar(out=m1[:], in0=rt[:, :, 0], scalar1=mebc[:, 0:1],
                                    scalar2=None, op0=OP.is_equal)
            m2 = sb.tile([P, NTILES], f32, tag="m2")
            nc.vector.tensor_scalar(out=m2[:], in0=rt[:, :, 1], scalar1=mebc[:, 0:1],
                                    scalar2=None, op0=OP.is_equal)
            msk = sb.tile([P, NTILES], f32, tag="msk")
            nc.vector.tensor_add(out=msk[:], in0=m1[:], in1=m2[:])

            iw = sb.tile([P, NTILES, 2], f32, tag="iw")
            nc.gpsimd.iota(out=iw[:, :, 0], pattern=[[P, NTILES]], base=0,
                           channel_multiplier=1,
                           allow_small_or_imprecise_dtypes=True)
            wa = sb.tile([P, NTILES], f32, tag="wa")
            nc.vector.tensor_mul(out=wa[:], in0=m1[:], in1=rt[:, :, 2])
            wb = sb.tile([P, NTILES], f32, tag="wb")
            nc.vector.tensor_mul(out=wb[:], in0=m2[:], in1=rt[:, :, 3])
            nc.vector.tensor_add(out=iw[:, :, 1], in0=wa[:], in1=wb[:])

            # within-column exclusive prefix (over partitions)
            pp_ps = ps.tile([P, NTILES], f32, tag="mm", bufs=6)
            nc.tensor.matmul(out=pp_ps[:], lhsT=su128_sb[:], rhs=msk[:],
                             skip_group_check=True)
            # column sums -> [1, 32]
            cs_ps = ps.tile([1, NTILES], f32, tag="mm", bufs=6)
            nc.tensor.matmul(out=cs_ps[:], lhsT=ones128_1[:], rhs=msk[:],
                             skip_group_check=True)
            cs_sb = sb.tile([1, NTILES], f32, tag="cs")
            nc.vector.tensor_copy(out=cs_sb[:], in_=cs_ps[:])
            # transpose to [32, 1]
            csT_ps = ps.tile([NTILES, 1], f32, tag="mm", bufs=6)
            nc.tensor.matmul(out=csT_ps[:], lhsT=cs_sb[:], rhs=one1[:],
                             skip_group_check=True)
            csT_sb = sb.tile([NTILES, 1], f32, tag="csT")
            nc.vector.tensor_copy(out=csT_sb[:], in_=csT_ps[:])
            # exclusive prefix over the 32 column sums
            cp_ps = ps.tile([NTILES, 1], f32, tag="mm", bufs=6)
            nc.tensor.matmul(out=cp_ps[:], lhsT=su32_sb[:], rhs=csT_sb[:],
                             skip_group_check=True)
            cp_sb = sb.tile([NTILES, 1], f32, tag="cp")
            nc.vector.tensor_copy(out=cp_sb[:], in_=cp_ps[:])
            # transpose back to a row [1, 32]
            cpr_ps = ps.tile([1, NTILES], f32, tag="mm", bufs=6)
            nc.tensor.transpose(out=cpr_ps[:], in_=cp_sb[:],
                                identity=id128f[:NTILES, :NTILES])
            cpr_sb = sb.tile([1, NTILES], f32, tag="cpr")
            nc.vector.tensor_copy(out=cpr_sb[:], in_=cpr_ps[:])
            # broadcast over partitions
            cpb_ps = ps.tile([P, NTILES], f32, tag="mm", bufs=6)
            nc.tensor.matmul(out=cpb_ps[:], lhsT=ones1_128[:], rhs=cpr_sb[:],
                             skip_group_check=True)
            cpb_sb = sb.tile([P, NTILES], f32, tag="cpb")
            nc.vector.tensor_copy(out=cpb_sb[:], in_=cpb_ps[:])

            pos = sb.tile([P, NTILES], f32, tag="pos")
            nc.vector.tensor_add(out=pos[:], in0=pp_ps[:], in1=cpb_sb[:])
            # non-selected tokens -> slot CAP (dump row); clamp overflow to CAP
            pos1 = sb.tile([P, NTILES], f32, tag="pos1")
            nc.vector.tensor_scalar(out=pos1[:], in0=pos[:], scalar1=float(CAP),
                                    scalar2=None, op0=OP.subtract)
            pos2 = sb.tile([P, NTILES], f32, tag="pos2")
            nc.vector.tensor_mul(out=pos2[:], in0=pos1[:], in1=msk[:])
            pos3 = sb.tile([P, NTILES], f32, tag="pos3")
            nc.vector.tensor_scalar(out=pos3[:], in0=pos2[:], scalar1=float(CAP),
                                    scalar2=float(CAP), op0=OP.add, op1=OP.min)
            posi = sb.tile([P, NTILES], i32, tag="posi")
            nc.vector.tensor_copy(out=posi[:], in_=pos3[:])

            # ---------------- phase D: compact (id, gate) via scatter -------
            pf = sb.tile([P, 2], f32, tag="pf")
            nc.vector.memset(pf[:, 0:1], float(DUMP))
            nc.vector.memset(pf[:, 1:2], 0.0)
            for j in range(11):
                nc.sync.dma_start(out=pk[j * P:(j + 1) * P, :], in_=pf[:])
            if "scatter" not in skip:
                for i in range(NTILES):
                    nc.gpsimd.indirect_dma_start(
                        out=pk[:],
                        out_offset=IndirectOffsetOnAxis(ap=posi[:, i:i + 1], axis=0),
                        in_=iw[:, i, :],
                        in_offset=None,
                    )
            pkt = sb.tile([P, NBLK, 2], f32, tag="pkt")
            nc.sync.dma_start(
                out=pkt[:], in_=pk[0:CAP, :].rearrange("(b p) c -> p b c", p=P)
            )
            idsi = sb.tile([P, NBLK], i32, tag="idsi")
            nc.vector.tensor_copy(out=idsi[:], in_=pkt[:, :, 0])
            nc.sync.dma_start(
                out=yids_d[:].rearrange("(b p) c -> p b c", p=P),
                in_=idsi[:, :, None],
            )

            # ---------------- phase F: gather x rows + transpose ------------
            xeT = sb.tile([P, KD, CAP], bf16, tag="xeT")
            for b in range(NBLK):
                xg = sb.tile([P, D], bf16, tag="xbuf", bufs=2)
                if "gather" in skip:
                    nc.sync.dma_start(out=xg[:], in_=xb_d[b * P:(b + 1) * P, :])
                else:
                    nc.gpsimd.indirect_dma_start(
                        out=xg[:],
                        out_offset=None,
                        in_=xb_d[:],
                        in_offset=IndirectOffsetOnAxis(ap=idsi[:, b:b + 1], axis=0),
                    )
                for k in range(KD):
                    tpb = ps.tile([P, P], bf16, tag="tp", bufs=2)
                    nc.tensor.transpose(
                        out=tpb[:], in_=xg[:, k * P:(k + 1) * P],
                        identity=id128b[:],
                    )
                    nc.scalar.copy(out=xeT[:, k, b * P:(b + 1) * P], in_=tpb[:])

            # ---------------- phase G: GEMM1 (h = relu(x W1 + b1)) ----------
            hT = sb.tile([P, KH, CAP], bf16, tag="hT")
            for m in range(MH):
                w1s = sb.tile([P, KD, P], bf16, tag="wstrip", bufs=2)
                nc.sync.dma_start(out=w1s[:], in_=w1_d[m])
                h_ps = [
                    ps.tile([P, W], f32, tag="mm", bufs=6, name=f"hps{m}_{cb}")
                    for cb, (c0, W) in enumerate(CB)
                ]
                for k in range(KD):
                    for cb, (c0, W) in enumerate(CB):
                        nc.tensor.matmul(
                            out=h_ps[cb][:], lhsT=w1s[:, k, :],
                            rhs=xeT[:, k, c0:c0 + W],
                            start=(k == 0), stop=(k == KD - 1),
                            skip_group_check=True,
                        )
                for cb, (c0, W) in enumerate(CB):
                    nc.scalar.activation(
                        out=hT[:, m, c0:c0 + W], in_=h_ps[cb][:], func=AF.Relu,
                        bias=b1_sb[:, m:m + 1], scale=1.0,
                    )

            # ---------------- phase H/I: GEMM2 + gate-scale + emit ----------
            for cb, (c0, W) in enumerate(CB):
                nt4 = W // P
                for mo in range(MD):
                    w2s = sb.tile([P, KH, P], bf16, tag="wstrip", bufs=2)
                    nc.sync.dma_start(out=w2s[:], in_=w2_d[mo])
                    y_ps = ps.tile([P, W], f32, tag="mm", bufs=6)
                    for k in range(KH):
                        nc.tensor.matmul(
                            out=y_ps[:], lhsT=w2s[:, k, :],
                            rhs=hT[:, k, c0:c0 + W],
                            start=(k == 0), stop=(k == KH - 1),
                            skip_group_check=True,
                        )
                    yT = sb.tile([P, W], f32, tag="yt", bufs=2)
                    nc.scalar.activation(
                        out=yT[:], in_=y_ps[:], func=AF.Identity,
                        bias=b2_sb[:, mo:mo + 1], scale=1.0,
                    )
                    for t4 in range(nt4):
                        bb = cb * 4 + t4
                        tpy = ps.tile([P, P], f32, tag="tp", bufs=2)
                        nc.tensor.transpose(
                            out=tpy[:], in_=yT[:, t4 * P:(t4 + 1) * P],
                            identity=id128f[:],
                        )
                        ystg = sb.tile([P, P], f32, tag="ystg", bufs=3)
                        nc.vector.tensor_scalar(
                            out=ystg[:], in0=tpy[:],
                            scalar1=pkt[:, bb, 1:2], scalar2=None, op0=OP.mult,
                        )
                        nc.sync.dma_start(
                            out=ycomp_d[bb * P:(bb + 1) * P, mo * P:(mo + 1) * P],
                            in_=ystg[:],
                        )

    return nc


def _get_prog():
    global _PROG
    if _PROG is None:
        _PROG = build_program()
    return _PROG


# ---------------------------------------------------------------------------
# Host-side sharding / unsharding
# ---------------------------------------------------------------------------
def make_in_maps(x, Wg, bg, W1, b1, W2, b2):
    bf16 = ml_dtypes.bfloat16
    x = np.ascontiguousarray(np.asarray(x, dtype=np.float32))
    Wg = np.ascontiguousarray(np.asarray(Wg, dtype=np.float32))
    bg = np.asarray(bg, dtype=np.float32).reshape(1, E)
    W1 = np.asarray(W1, dtype=np.float32)
    b1 = np.asarray(b1, dtype=np.float32)
    W2 = np.asarray(W2, dtype=np.float32)
    b2 = np.asarray(b2, dtype=np.float32)

    xb = np.concatenate([x, np.zeros((1, D), np.float32)], axis=0).astype(bf16)
    su128 = (np.arange(P)[:, None] < np.arange(P)[None, :]).astype(np.float32)
    su32 = (np.arange(32)[:, None] < np.arange(32)[None, :]).astype(np.float32)

    in_maps = []
    for e in range(N_CORES):
        w1e = W1[e].reshape(KD, P, MH, P).transpose(2, 1, 0, 3)  # [MH,P(d),KD,P(h)]
        w2e = W2[e].reshape(KH, P, MD, P).transpose(2, 1, 0, 3)  # [MD,P(h),KH,P(d)]
        in_maps.append({
            "xs": np.ascontiguousarray(x[e * SHARD:(e + 1) * SHARD]),
            "xb": xb,
            "wg": Wg,
            "bg": bg,
            "myexp": np.array([[float(e)]], np.float32),
            "w1t": np.ascontiguousarray(w1e).astype(bf16),
            "w2t": np.ascontiguousarray(w2e).astype(bf16),
            "b1r": np.ascontiguousarray(b1[e].reshape(MH, P).T),
            "b2r": np.ascontiguousarray(b2[e].reshape(MD, P).T),
            "su128": su128,
            "su32": su32,
        })
    return in_maps


def assemble_outputs(results):
    probs = np.concatenate([np.asarray(r["probs"]) for r in results], axis=0)
    topk = np.concatenate([np.asarray(r["topk"]) for r in results], axis=0)
    out = np.zeros((B + 1, D), np.float32)
    for r in results:
        ids = np.asarray(r["yids"]).reshape(-1)
        out[ids] += np.asarray(r["ycomp"], dtype=np.float32)
    return out[:B], probs, topk.astype(np.int32)


def kernel(x, Wg, bg, W1, b1, W2, b2):
    from concourse.bass_utils import run_bass_kernel_spmd

    nc = _get_prog()
    in_maps = make_in_maps(x, Wg, bg, W1, b1, W2, b2)
    res = run_bass_kernel_spmd(nc, in_maps, core_ids=list(range(N_CORES)))
    return assemble_outputs(res.results)
